# revision 1
# baseline (speedup 1.0000x reference)
"""Trainium2 Bass kernel for nn_MixedSparseGatedMLP (LoRA-augmented gated MLP).

Math (reference):
    y1 = x @ Wg + (x @ Ag) @ Bg
    y2 = x @ Wu + (x @ Au) @ Bu
    x3 = relu(y1) * y2
    y3 = x3 @ Wd + (x3 @ Ad) @ Bd

Strategy:
  - Fold the rank-16 LoRA factors into the dense weights on the host
    (exact fp32 algebra): Wg_eff = Wg + Ag@Bg, etc.  The device kernel is
    then a plain gated MLP with three dense matmuls.
  - Tensor-parallel over the intermediate dim I across 8 NeuronCores:
    each core owns I/8 columns of Wg_eff/Wu_eff and I/8 rows of Wd_eff,
    processes ALL tokens, and produces a partial [NTOK, H] output.
    Partials are summed on the host (gather/unshard step).
  - bf16 operands, fp32 PSUM accumulation, fp32 partial outputs.
  - All DRAM layouts are pre-tiled on the host so every DMA is a linear
    (or near-linear) copy into the exact SBUF layout the matmuls need.
"""

import os
import sys

for _p in ("/opt/trn_rl_repo", "/root/.axon_site/_ro/trn_rl_repo"):
    if os.path.isdir(_p) and _p not in sys.path:
        sys.path.append(_p)

import numpy as np
import ml_dtypes

# Problem shapes (hardcoded per contract)
B, S, H, I, R = 2, 2048, 4096, 11008, 16
NTOK = B * S              # 4096 tokens
NCORES = 8
IPAD = 11264              # I padded to 88*128 so it splits 8 ways into 128-chunks
IS = IPAD // NCORES       # 1408 intermediate columns per core
C = IS // 128             # 11 i-chunks per core
K = H // 128              # 32 h-chunks
TB = 512                  # token block
NB = NTOK // TB           # 8 token blocks
MT = TB // 128            # 4 token m-tiles per block
NH = H // 512             # 8 output n-tiles

BF16 = ml_dtypes.bfloat16

# set by test.py for profiling; harness path leaves these as-is
TRACE = False
LAST_EXEC_TIME_NS = None
LAST_RESULTS = None


def _build_nc():
    import concourse.bacc as bacc
    import concourse.mybir as mybir
    import concourse.tile as tile

    bf16 = mybir.dt.bfloat16
    f32 = mybir.dt.float32

    nc = bacc.Bacc("TRN2", target_bir_lowering=False, debug=False)

    # DRAM parameters (host pre-tiled layouts; see kernel() for the math)
    x = nc.declare_dram_parameter("x", [NB, 128, K * TB], bf16, isOutput=False)
    wg = nc.declare_dram_parameter("wg", [C, 128, K * 128], bf16, isOutput=False)
    wu = nc.declare_dram_parameter("wu", [C, 128, K * 128], bf16, isOutput=False)
    wd = nc.declare_dram_parameter("wd", [C, 128, H], bf16, isOutput=False)
    out = nc.declare_dram_parameter("out", [NTOK, H], f32, isOutput=True)

    with tile.TileContext(nc) as tc:
        with tc.tile_pool(name="xp", bufs=1) as xp, \
             tc.tile_pool(name="wp", bufs=4) as wp, \
             tc.tile_pool(name="wdp", bufs=1) as wdp, \
             tc.tile_pool(name="x3p", bufs=2) as x3p, \
             tc.tile_pool(name="rp", bufs=2) as rp, \
             tc.tile_pool(name="op", bufs=4) as op, \
             tc.tile_pool(name="pgp", bufs=3, space="PSUM") as pgp, \
             tc.tile_pool(name="pup", bufs=3, space="PSUM") as pup, \
             tc.tile_pool(name="pdp", bufs=2, space="PSUM") as pdp:

            wdt = []
            KG = 4            # k-groups per block (split DMAs so the first
            KS = K // KG      # matmuls gate on ~1MB, not the full 4MB)
            # wd chunks issued per m-iteration of block 0, back-loaded so
            # the startup ramp (weight stream + x) gets the full HBM BW
            WD_SCHED = [0, 0, 0, 0, 2, 2, 2, 2, 1, 1, 1]

            # PE warmup: dependency-free matmuls run during the initial DMA
            # wait and lift the HAM clock gate to 8/8 before real work.
            warm_in = rp.tile([128, TB], bf16, tag="warm")
            nc.any.memset(warm_in, 0.0)
            warm_ps = pdp.tile([128, TB], f32, tag="pd")
            for _ in range(52):
                nc.tensor.matmul(warm_ps, warm_in[:, 0:128], warm_in,
                                 start=True, stop=True)

            def w_tile(src, m, eng=None):
                # monolithic 1MB weight DMAs: large transfers fan out over
                # more DMA engines and sustain ~2x the per-queue bandwidth
                # of 256KB tiles (k-split weights starved the block-0 ramp)
                t = wp.tile([128, K * 128], bf16, tag="w")
                (eng or nc.sync).dma_start(t, src[m])
                return t

            for b in range(NB):
                # x block, split into KG tiles: [128 h-in-chunk, (k, t)] bf16
                xbg = []
                w0 = None
                if b == 0:
                    # Startup ramp is HBM-bound with no previous block to
                    # hide under.  Split the ~6MB critical mass across both
                    # DGE paths, emitted in need-time order:
                    #   HWDGE: wg0@11us, x-g0@11, x-g1@13
                    #   SWDGE: x-g2@14.5, x-g3@16, wu0@18
                    wgt0 = w_tile(wg, 0)
                    for gi in range(KG):
                        t = xp.tile([128, KS * TB], bf16, tag=f"xb{gi}")
                        eng = nc.sync if gi < 2 else nc.gpsimd
                        eng.dma_start(t, x[b][:, gi * KS * TB:(gi + 1) * KS * TB])
                        xbg.append(t)
                    wut0 = w_tile(wu, 0, nc.gpsimd)
                    w0 = (wgt0, wut0)
                else:
                    for gi in range(KG):
                        t = xp.tile([128, KS * TB], bf16, tag=f"xb{gi}")
                        nc.sync.dma_start(t, x[b][:, gi * KS * TB:(gi + 1) * KS * TB])
                        xbg.append(t)

                # x3^T for this block: [128 i-in-chunk, (c, t)] bf16
                x3 = x3p.tile([128, C * TB], bf16, tag="x3")

                # ---- gate / up projections + gating, per i-chunk m ----
                for m in range(C):
                    if m == 0 and w0 is not None:
                        wgt, wut = w0
                    else:
                        # block 0 has no previous down-phase to build DMA
                        # lead under; split its weight stream evenly across
                        # both DGE paths (SWDGE is otherwise idle)
                        wgt = w_tile(wg, m)
                        wut = w_tile(wu, m, nc.gpsimd if b == 0 else None)

                    g = pgp.tile([128, TB], f32, tag="pg")
                    u = pup.tile([128, TB], f32, tag="pu")
                    for k in range(K):
                        gi, kk = divmod(k, KS)
                        nc.tensor.matmul(
                            g,
                            wgt[:, k * 128:(k + 1) * 128],
                            xbg[gi][:, kk * TB:(kk + 1) * TB],
                            start=(k == 0), stop=(k == K - 1),
                        )
                    for k in range(K):
                        gi, kk = divmod(k, KS)
                        nc.tensor.matmul(
                            u,
                            wut[:, k * 128:(k + 1) * 128],
                            xbg[gi][:, kk * TB:(kk + 1) * TB],
                            start=(k == 0), stop=(k == K - 1),
                        )
                    # x3 = relu(g) * u ; DVE may read only one PSUM input,
                    # so relu lands in SBUF via ACT first.
                    r = rp.tile([128, TB], bf16, tag="r")
                    nc.scalar.activation(r, g, mybir.ActivationFunctionType.Relu)
                    nc.vector.tensor_mul(x3[:, m * TB:(m + 1) * TB], r, u)

                    if b == 0:
                        # Wd_eff stays SBUF-resident for the whole kernel
                        # (11 x 1MB).  Preload on the software-DGE path,
                        # back-loaded per WD_SCHED: all chunks must land by
                        # block 0's down phase (~185us), but issuing them
                        # early floods HBM during the startup ramp.
                        for _ in range(WD_SCHED[m]):
                            c = len(wdt)
                            t = wdp.tile([128, H], bf16, tag=f"wd{c}")
                            nc.gpsimd.dma_start(t, wd[c])
                            wdt.append(t)

                # ---- down projection: out[tok, h] partial ----
                for mt in range(MT):
                    for n in range(NH):
                        d = pdp.tile([128, 512], f32, tag="pd")
                        for c in range(C):
                            nc.tensor.matmul(
                                d,
                                x3[:, c * TB + mt * 128: c * TB + (mt + 1) * 128],
                                wdt[c][:, n * 512:(n + 1) * 512],
                                start=(c == 0), stop=(c == C - 1),
                            )
                        o = op.tile([128, 512], f32, tag="o")
                        nc.scalar.copy(o, d)
                        row = b * TB + mt * 128
                        # store via SWDGE: keeps HWDGE free for the
                        # x/weight prefetches that gate the next block.
                        # Last block: HWDGE is idle, and its lower latency
                        # trims the kernel tail.
                        st = nc.sync if b == NB - 1 else nc.gpsimd
                        st.dma_start(
                            out[row:row + 128, n * 512:(n + 1) * 512], o
                        )

    nc.compile()
    return nc


def _prep_inputs(x1, w_gate, w_gate_lora_a, w_gate_lora_b,
                 w_up, w_up_lora_a, w_up_lora_b,
                 w_down, w_down_lora_a, w_down_lora_b):
    """Fold LoRA, pad I, shard per core, and pre-tile DRAM layouts."""
    f32 = np.float32
    x1 = np.asarray(x1, f32)
    wg_eff = np.asarray(w_gate, f32) + np.asarray(w_gate_lora_a, f32) @ np.asarray(w_gate_lora_b, f32)
    wu_eff = np.asarray(w_up, f32) + np.asarray(w_up_lora_a, f32) @ np.asarray(w_up_lora_b, f32)
    wd_eff = np.asarray(w_down, f32) + np.asarray(w_down_lora_a, f32) @ np.asarray(w_down_lora_b, f32)

    wg_p = np.zeros((H, IPAD), f32); wg_p[:, :I] = wg_eff
    wu_p = np.zeros((H, IPAD), f32); wu_p[:, :I] = wu_eff
    wd_p = np.zeros((IPAD, H), f32); wd_p[:I, :] = wd_eff

    # x tile layout: x_tiled[b, p, k, t] = x2d[b*TB + t, k*128 + p]
    x2d = x1.reshape(NTOK, H)
    x_tiled = np.ascontiguousarray(
        x2d.reshape(NB, TB, K, 128).transpose(0, 3, 2, 1)
    ).astype(BF16).reshape(NB, 128, K * TB)

    in_maps = []
    for ci in range(NCORES):
        sl = slice(ci * IS, (ci + 1) * IS)
        # wg tile layout: [m, p, k, i] = wg_p[k*128+p, ci*IS + m*128 + i]
        wgc = np.ascontiguousarray(
            wg_p[:, sl].reshape(K, 128, C, 128).transpose(2, 1, 0, 3)
        ).astype(BF16).reshape(C, 128, K * 128)
        wuc = np.ascontiguousarray(
            wu_p[:, sl].reshape(K, 128, C, 128).transpose(2, 1, 0, 3)
        ).astype(BF16).reshape(C, 128, K * 128)
        # wd tile layout: [c, p, h] = wd_p[ci*IS + c*128 + p, h]
        wdc = wd_p[sl, :].reshape(C, 128, H).astype(BF16)
        in_maps.append({"x": x_tiled, "wg": wgc, "wu": wuc, "wd": wdc})
    return in_maps


def _emulate(in_maps):
    """Numpy emulation of the device math (bf16 operands, fp32 accum).
    Validates the host-side tilings and predicts the on-device accuracy."""
    f32 = np.float32
    acc = np.zeros((NTOK, H), f32)
    # reconstruct x2d (bf16-rounded) from the tiled layout
    xt = in_maps[0]["x"].reshape(NB, 128, K, TB)
    x2d = xt.transpose(0, 3, 2, 1).reshape(NTOK, H).astype(f32)
    for m in in_maps:
        wgc = m["wg"].reshape(C, 128, K, 128)
        wg2 = wgc.transpose(2, 1, 0, 3).reshape(H, IS).astype(f32)
        wuc = m["wu"].reshape(C, 128, K, 128)
        wu2 = wuc.transpose(2, 1, 0, 3).reshape(H, IS).astype(f32)
        wd2 = m["wd"].reshape(IS, H).astype(f32)
        y1 = x2d @ wg2
        y2 = x2d @ wu2
        r = np.maximum(y1, 0).astype(BF16).astype(f32)
        x3 = (r * y2).astype(BF16).astype(f32)
        acc += x3 @ wd2
    return acc.reshape(B, S, H)


def kernel(**inputs):
    global LAST_EXEC_TIME_NS, LAST_RESULTS
    in_maps = _prep_inputs(**inputs)

    if os.environ.get("KERNEL_EMULATE"):
        return _emulate(in_maps)

    from concourse.bass_utils import run_bass_kernel_spmd

    nc = _build_nc()
    res = run_bass_kernel_spmd(nc, in_maps, list(range(NCORES)), trace=TRACE)
    LAST_EXEC_TIME_NS = res.exec_time_ns
    LAST_RESULTS = res

    acc = np.zeros((NTOK, H), np.float32)
    for r in res.results:
        acc += r["out"]
    return acc.reshape(B, S, H)



# revision 5
# speedup vs baseline: 1.0162x; 1.0162x over previous
"""Trainium2 Bass kernel for nn_MixedSparseGatedMLP (LoRA-augmented gated MLP).

Math (reference):
    y1 = x @ Wg + (x @ Ag) @ Bg
    y2 = x @ Wu + (x @ Au) @ Bu
    x3 = relu(y1) * y2
    y3 = x3 @ Wd + (x3 @ Ad) @ Bd

Strategy:
  - Fold the rank-16 LoRA factors into the dense weights on the host
    (exact fp32 algebra): Wg_eff = Wg + Ag@Bg, etc.  The device kernel is
    then a plain gated MLP with three dense matmuls.
  - Padding-free hybrid sharding over the intermediate dim I = 86*128:
    each core OWNS 10 i-chunks (processed for all 4096 tokens) and the 6
    leftover chunks are processed data-parallel: every core handles them
    for its OWN 512-token block only ("shared phase", fed by a per-core
    x_shared input).  Per-core work = 8 blocks*10 + 6 = 86 chunk-blocks,
    exactly 1/8 of the unpadded problem (the old I-padding to 88 chunks
    wasted 2.3% of PE time).
  - bf16 operands, fp32 PSUM accumulation, bf16 partial outputs (the
    8-way host reduction in fp32 keeps the rounding error ~4e-3).
  - All DRAM layouts are pre-tiled on the host so every DMA is a linear
    (or near-linear) copy into the exact SBUF layout the matmuls need.
  - Ramp: DMA flow starts ~9us into the kernel (fixed DGE latency) at
    ~330-430 GB/s aggregate.  Block 0 chunk 0's weights are split into
    256KB pieces and x into 512KB k-groups, need-ordered across the
    HWDGE (sync) and SWDGE (gpsimd) paths, so the first real matmul
    gates on ~0.75MB instead of ~6MB.  N=128 warmup matmuls keep the
    PE busy (and the HAM clock-gate warm) until the data lands.
"""

import os
import sys

for _p in ("/opt/trn_rl_repo", "/root/.axon_site/_ro/trn_rl_repo"):
    if os.path.isdir(_p) and _p not in sys.path:
        sys.path.append(_p)

import numpy as np
import ml_dtypes

# Problem shapes (hardcoded per contract)
B, S, H, I, R = 2, 2048, 4096, 11008, 16
NTOK = B * S              # 4096 tokens
NCORES = 8
CI = I // 128             # 86 i-chunks total (no padding: 86*128 == I)
C = 10                    # i-chunks OWNED per core (all tokens)
CS = CI - NCORES * C      # 6 leftover i-chunks, data-parallel over blocks
IS = C * 128              # 1280 owned intermediate columns per core
K = H // 128              # 32 h-chunks
TB = 512                  # token block
NB = NTOK // TB           # 8 token blocks
MT = TB // 128            # 4 token m-tiles per block
NH = H // 512             # 8 output n-tiles
KG = 8                    # x k-groups per block (512KB DMAs)
KS = K // KG              # 4 k-chunks per group

BF16 = ml_dtypes.bfloat16

# set by test.py for profiling; harness path leaves these as-is
TRACE = False
LAST_EXEC_TIME_NS = None
LAST_RESULTS = None


def _build_nc():
    import concourse.bacc as bacc
    import concourse.mybir as mybir
    import concourse.tile as tile

    bf16 = mybir.dt.bfloat16
    f32 = mybir.dt.float32

    nc = bacc.Bacc("TRN2", target_bir_lowering=False, debug=False)

    # DRAM parameters (host pre-tiled layouts; see kernel() for the math)
    x = nc.declare_dram_parameter("x", [NB, 128, K * TB], bf16, isOutput=False)
    xs = nc.declare_dram_parameter("xs", [128, K * TB], bf16, isOutput=False)
    wg = nc.declare_dram_parameter("wg", [C, 128, K * 128], bf16, isOutput=False)
    wu = nc.declare_dram_parameter("wu", [C, 128, K * 128], bf16, isOutput=False)
    wgs = nc.declare_dram_parameter("wgs", [CS, 128, K * 128], bf16, isOutput=False)
    wus = nc.declare_dram_parameter("wus", [CS, 128, K * 128], bf16, isOutput=False)
    wd = nc.declare_dram_parameter("wd", [C, 128, H], bf16, isOutput=False)
    wds = nc.declare_dram_parameter("wds", [CS, 128, H], bf16, isOutput=False)
    out = nc.declare_dram_parameter("out", [NTOK, H], bf16, isOutput=True)
    outs = nc.declare_dram_parameter("outs", [TB, H], bf16, isOutput=True)

    with tile.TileContext(nc) as tc:
        with tc.tile_pool(name="xp", bufs=1) as xp, \
             tc.tile_pool(name="wp0", bufs=1) as wp0, \
             tc.tile_pool(name="wp", bufs=4) as wp, \
             tc.tile_pool(name="wdp", bufs=1) as wdp, \
             tc.tile_pool(name="wdsp", bufs=2) as wdsp, \
             tc.tile_pool(name="x3p", bufs=2) as x3p, \
             tc.tile_pool(name="rp", bufs=2) as rp, \
             tc.tile_pool(name="op", bufs=4) as op, \
             tc.tile_pool(name="pgp", bufs=3, space="PSUM") as pgp, \
             tc.tile_pool(name="pup", bufs=3, space="PSUM") as pup, \
             tc.tile_pool(name="pdp", bufs=2, space="PSUM") as pdp:

            wdt = []
            # wd chunks issued per m-iteration of block 0, back-loaded so
            # the startup ramp (weight stream + x) gets the full HBM BW
            WD_SCHED = [0, 0, 0, 0, 2, 2, 2, 2, 1, 1]

            # PE warmup: dependency-free N=128 matmuls run during the
            # fixed ~9us DMA-start latency + first-piece transfer; they
            # lift the HAM clock gate to 8/8 and keep the PE busy until
            # the first real operands land (~11-13us).
            warm_in = rp.tile([128, TB], bf16, tag="warm")
            nc.any.memset(warm_in, 0.0)
            warm_ps = pdp.tile([128, 128], f32, tag="pd")
            for _ in range(150):
                nc.tensor.matmul(warm_ps, warm_in[:, 0:128],
                                 warm_in[:, 0:128], start=True, stop=True)

            def w_tile(src, m, eng=None):
                # monolithic 1MB weight DMAs: large transfers fan out over
                # more DMA engines and sustain ~2x the per-queue bandwidth
                # of 256KB tiles
                t = wp.tile([128, K * 128], bf16, tag="w")
                (eng or nc.sync).dma_start(t, src[m])
                return t

            def gate_up_chunk(wgt_pieces, wut_pieces, xbg, x3, m):
                # wgt/wut given as lists of (tile, k0) pieces covering k
                g = pgp.tile([128, TB], f32, tag="pg")
                u = pup.tile([128, TB], f32, tag="pu")
                for pieces, ps in ((wgt_pieces, g), (wut_pieces, u)):
                    for t, k0, kn in pieces:
                        for kk in range(kn):
                            k = k0 + kk
                            gi, kg = divmod(k, KS)
                            nc.tensor.matmul(
                                ps,
                                t[:, kk * 128:(kk + 1) * 128],
                                xbg[gi][:, kg * TB:(kg + 1) * TB],
                                start=(k == 0), stop=(k == K - 1),
                            )
                # x3 = relu(g) * u ; DVE may read only one PSUM input,
                # so relu lands in SBUF via ACT first.
                r = rp.tile([128, TB], bf16, tag="r")
                nc.scalar.activation(r, g, mybir.ActivationFunctionType.Relu)
                nc.vector.tensor_mul(x3[:, m * TB:(m + 1) * TB], r, u)

            for b in range(NB):
                # x block in KG groups: [128 h-in-chunk, (k, t)] bf16
                xbg = [None] * KG
                w0 = None
                if b == 0:
                    # Startup ramp: interleave 256KB weight pieces and
                    # 512KB x groups across both DGE paths in need-time
                    # order, so the first matmul gates on ~0.75MB.
                    def xg(gi, eng):
                        t = xp.tile([128, KS * TB], bf16, tag=f"xb{gi}")
                        eng.dma_start(t, x[b][:, gi * KS * TB:(gi + 1) * KS * TB])
                        xbg[gi] = t

                    def wpiece(src, j, eng):
                        t = wp0.tile([128, 8 * 128], bf16, tag=f"{src}p{j}")
                        s = wg if src == "g" else wu
                        eng.dma_start(t, s[0][:, j * 8 * 128:(j + 1) * 8 * 128])
                        return (t, j * 8, 8)

                    wgt0 = []
                    wut0 = []
                    # sync (HWDGE): wg0a xb1 wg0b xb3 wg0c xb5 wg0d xb7
                    # gpsimd (SWDGE): xb0 xb2 xb4 xb6 wu0a..d
                    wgt0.append(wpiece("g", 0, nc.sync))
                    xg(0, nc.gpsimd)
                    xg(1, nc.sync)
                    xg(2, nc.gpsimd)
                    wgt0.append(wpiece("g", 1, nc.sync))
                    xg(4, nc.gpsimd)
                    xg(3, nc.sync)
                    wgt0.append(wpiece("g", 2, nc.sync))
                    xg(6, nc.gpsimd)
                    xg(5, nc.sync)
                    wut0.append(wpiece("u", 0, nc.gpsimd))
                    wgt0.append(wpiece("g", 3, nc.sync))
                    wut0.append(wpiece("u", 1, nc.gpsimd))
                    xg(7, nc.sync)
                    wut0.append(wpiece("u", 2, nc.gpsimd))
                    wut0.append(wpiece("u", 3, nc.gpsimd))
                    w0 = (wgt0, wut0)
                else:
                    for gi in range(KG):
                        t = xp.tile([128, KS * TB], bf16, tag=f"xb{gi}")
                        nc.sync.dma_start(t, x[b][:, gi * KS * TB:(gi + 1) * KS * TB])
                        xbg[gi] = t

                # x3^T for this block: [128 i-in-chunk, (c, t)] bf16
                x3 = x3p.tile([128, C * TB], bf16, tag="x3")

                # ---- gate / up projections + gating, per i-chunk m ----
                for m in range(C):
                    if m == 0 and w0 is not None:
                        wgt_p, wut_p = w0
                    else:
                        # block 0 has no previous down-phase to build DMA
                        # lead under; split its weight stream evenly across
                        # both DGE paths (SWDGE is otherwise idle)
                        wgt_p = [(w_tile(wg, m), 0, K)]
                        wut_p = [(w_tile(wu, m, nc.gpsimd if b == 0 else None), 0, K)]

                    gate_up_chunk(wgt_p, wut_p, xbg, x3, m)

                    if b == 0:
                        # Wd (own chunks) stays SBUF-resident for the whole
                        # kernel (10 x 1MB).  Preload on the software-DGE
                        # path, back-loaded per WD_SCHED: all chunks must
                        # land by block 0's down phase, but issuing them
                        # early floods HBM during the startup ramp.
                        for _ in range(WD_SCHED[m]):
                            c = len(wdt)
                            t = wdp.tile([128, H], bf16, tag=f"wd{c}")
                            nc.gpsimd.dma_start(t, wd[c])
                            wdt.append(t)

                # ---- down projection: out[tok, h] partial ----
                for mt in range(MT):
                    for n in range(NH):
                        d = pdp.tile([128, 512], f32, tag="pd")
                        for c in range(C):
                            nc.tensor.matmul(
                                d,
                                x3[:, c * TB + mt * 128: c * TB + (mt + 1) * 128],
                                wdt[c][:, n * 512:(n + 1) * 512],
                                start=(c == 0), stop=(c == C - 1),
                            )
                        o = op.tile([128, 512], bf16, tag="o")
                        nc.scalar.copy(o, d)
                        row = b * TB + mt * 128
                        # store via SWDGE: keeps HWDGE free for the
                        # x/weight prefetches that gate the next block.
                        nc.gpsimd.dma_start(
                            out[row:row + 128, n * 512:(n + 1) * 512], o
                        )

            # ---- shared phase: the 6 leftover i-chunks, this core's own
            # 512-token block only (fed by the per-core xs input). ----
            xsg = []
            for gi in range(KG):
                t = xp.tile([128, KS * TB], bf16, tag=f"xb{gi}")
                nc.sync.dma_start(t, xs[:, gi * KS * TB:(gi + 1) * KS * TB])
                xsg.append(t)

            x3s = x3p.tile([128, CS * TB], bf16, tag="x3")
            for m in range(CS):
                wgt_p = [(w_tile(wgs, m), 0, K)]
                wut_p = [(w_tile(wus, m), 0, K)]
                gate_up_chunk(wgt_p, wut_p, xsg, x3s, m)

            # shared down: n-outer so the wds weights stream through a
            # small pool of [128,512] slices instead of living resident.
            for n in range(NH):
                wdst = []
                for c in range(CS):
                    t = wdsp.tile([128, 512], bf16, tag=f"wds{c}")
                    nc.gpsimd.dma_start(t, wds[c][:, n * 512:(n + 1) * 512])
                    wdst.append(t)
                for mt in range(MT):
                    d = pdp.tile([128, 512], f32, tag="pd")
                    for c in range(CS):
                        nc.tensor.matmul(
                            d,
                            x3s[:, c * TB + mt * 128: c * TB + (mt + 1) * 128],
                            wdst[c],
                            start=(c == 0), stop=(c == CS - 1),
                        )
                    o = op.tile([128, 512], bf16, tag="o")
                    nc.scalar.copy(o, d)
                    # HWDGE is idle in the shared down phase, and its
                    # lower latency trims the kernel tail.
                    nc.sync.dma_start(
                        outs[mt * 128:(mt + 1) * 128, n * 512:(n + 1) * 512], o
                    )

    nc.compile()
    return nc


def _prep_inputs(x1, w_gate, w_gate_lora_a, w_gate_lora_b,
                 w_up, w_up_lora_a, w_up_lora_b,
                 w_down, w_down_lora_a, w_down_lora_b):
    """Fold LoRA, shard per core (10 own + 6 shared chunks), pre-tile."""
    f32 = np.float32
    x1 = np.asarray(x1, f32)
    wg_eff = np.asarray(w_gate, f32) + np.asarray(w_gate_lora_a, f32) @ np.asarray(w_gate_lora_b, f32)
    wu_eff = np.asarray(w_up, f32) + np.asarray(w_up_lora_a, f32) @ np.asarray(w_up_lora_b, f32)
    wd_eff = np.asarray(w_down, f32) + np.asarray(w_down_lora_a, f32) @ np.asarray(w_down_lora_b, f32)

    # x tile layout: x_tiled[b, p, k, t] = x2d[b*TB + t, k*128 + p]
    x2d = x1.reshape(NTOK, H)
    x_tiled = np.ascontiguousarray(
        x2d.reshape(NB, TB, K, 128).transpose(0, 3, 2, 1)
    ).astype(BF16).reshape(NB, 128, K * TB)

    def wgu_tile(w, sl, c):
        # [m, p, k, i] = w[k*128+p, sl.start + m*128 + i]
        return np.ascontiguousarray(
            w[:, sl].reshape(K, 128, c, 128).transpose(2, 1, 0, 3)
        ).astype(BF16).reshape(c, 128, K * 128)

    sh = slice(NCORES * IS, I)       # the 6 shared chunks
    wgs_t = wgu_tile(wg_eff, sh, CS)
    wus_t = wgu_tile(wu_eff, sh, CS)
    wds_t = wd_eff[sh, :].reshape(CS, 128, H).astype(BF16)

    in_maps = []
    for ci in range(NCORES):
        sl = slice(ci * IS, (ci + 1) * IS)
        in_maps.append({
            "x": x_tiled,
            "xs": x_tiled[ci],
            "wg": wgu_tile(wg_eff, sl, C),
            "wu": wgu_tile(wu_eff, sl, C),
            "wgs": wgs_t,
            "wus": wus_t,
            "wd": wd_eff[sl, :].reshape(C, 128, H).astype(BF16),
            "wds": wds_t,
        })
    return in_maps


def _emulate(in_maps):
    """Numpy emulation of the device math (bf16 operands, fp32 accum).
    Validates the host-side tilings and predicts the on-device accuracy."""
    f32 = np.float32
    acc = np.zeros((NTOK, H), f32)
    # reconstruct x2d (bf16-rounded) from the tiled layout
    xt = in_maps[0]["x"].reshape(NB, 128, K, TB)
    x2d = xt.transpose(0, 3, 2, 1).reshape(NTOK, H).astype(f32)

    def untile_wgu(wt, c):
        return wt.reshape(c, 128, K, 128).transpose(2, 1, 0, 3).reshape(H, c * 128).astype(f32)

    def mlp(x, wg2, wu2, wd2):
        y1 = x @ wg2
        y2 = x @ wu2
        r = np.maximum(y1, 0).astype(BF16).astype(f32)
        x3 = (r * y2).astype(BF16).astype(f32)
        return (x3 @ wd2).astype(BF16).astype(f32)

    for ci, m in enumerate(in_maps):
        acc += mlp(x2d, untile_wgu(m["wg"], C), untile_wgu(m["wu"], C),
                   m["wd"].reshape(IS, H).astype(f32))
        rows = slice(ci * TB, (ci + 1) * TB)
        acc[rows] += mlp(x2d[rows], untile_wgu(m["wgs"], CS),
                         untile_wgu(m["wus"], CS),
                         m["wds"].reshape(CS * 128, H).astype(f32))
    return acc.reshape(B, S, H)


def kernel(**inputs):
    global LAST_EXEC_TIME_NS, LAST_RESULTS
    in_maps = _prep_inputs(**inputs)

    if os.environ.get("KERNEL_EMULATE"):
        return _emulate(in_maps)

    from concourse.bass_utils import run_bass_kernel_spmd

    nc = _build_nc()
    res = run_bass_kernel_spmd(nc, in_maps, list(range(NCORES)), trace=TRACE)
    LAST_EXEC_TIME_NS = res.exec_time_ns
    LAST_RESULTS = res

    acc = np.zeros((NTOK, H), np.float32)
    for ci, r in enumerate(res.results):
        acc += r["out"].astype(np.float32)
        rows = slice(ci * TB, (ci + 1) * TB)
        acc[rows] += r["outs"].astype(np.float32)
    return acc.reshape(B, S, H)
